# revision 7
# baseline (speedup 1.0000x reference)
"""Trainium2 Bass kernel for nn_Aggregation0 (scatter_memory).

8 cores = 4 frames x 2 image-halves (SPMD, one program). The host pre-sorts
patch rows into destination order per core; patches are stored j-major
(j, i, c) so the fold matmuls stream contiguous 21-element runs.

Device pipeline per block (tops blocked [36,36,36,18,5] - small tail blocks
shorten the pipeline drain):
  fold: per (chunk, j, g mod 7) bf16 matmuls vs shifted-identity weights;
    the g-residue split keeps one matmul's out runs non-overlapping while
    the fold overlap accumulates across matmuls in PSUM (has_written).
  merge: DVE stitches the 6-row block overlap + 1/w normalize -> vtu (SBUF)
  vb: PE transpose to (row,ch)-major; ACT copies to SBUF as bf16
  pj: per (j, left-chunk) bf16 PE transpose back to column-partition layout
  assembly: DVE/ACT strided copies expand rows into patches (bf16)
  -> sequential bf16 store (host inverse-maps).
DMA: inputs + consts on the SP HWDGE ring, outputs + identw on the ACT ring
so the two streams don't FIFO-serialize.
"""
import sys
if '/opt/trn_rl_repo' not in sys.path:
    sys.path.insert(0, '/opt/trn_rl_repo')
import numpy as np

import concourse.bacc as bacc
import concourse.bass as bass
import concourse.mybir as mybir
import concourse.tile as tile
from concourse.bass_utils import run_bass_kernel_spmd

T, HP, WP = 4, 256, 256
PS = 7
NPOS = 250
P = NPOS * NPOS
BT = [36, 36, 36, 18, 5]     # tops per block
NB = len(BT)
SB = [sum(BT[:k]) for k in range(NB)]     # block start top
RW = 42                      # rows per vt/vtu tile (max B + 6)
RC = 3 * RW                  # 126 (rl, c) columns
GW = 131
NREAL = GW * NPOS            # 32750
F32 = mybir.dt.float32
BF16 = mybir.dt.bfloat16
D = 147
# out-tops per block k: contiguous partition of [0, 130]
OT = [(max(0, SB[k] - 6),
       (SB[k + 1] - 7) if k + 1 < NB else 130) for k in range(NB)]
SIN = sum(BT) * 2            # 262 input slots of [128, 147]
SOUT = sum(t[1] - t[0] + 1 for t in OT) * 2   # 262 output slots
IN_W = SIN * D
OUT_W = SOUT * D

# device patch element order is (j, i, ch); COLPERM maps back to (ch, i, j)
COLPERM = np.zeros(147, np.int64)
PELEM = np.zeros(147, np.int64)
for _c in range(3):
    for _i in range(7):
        for _j in range(7):
            COLPERM[_c * 49 + _i * 7 + _j] = _j * 21 + _i * 3 + _c
            PELEM[_j * 21 + _i * 3 + _c] = _c * 49 + _i * 7 + _j


def _cntf(z):
    z = np.asarray(z, np.float64)
    return np.minimum(6, z) - np.maximum(0, z - 249) + 1


def _host_prep_core(x, nlInds, c):
    f, h = c >> 1, c & 1
    g0 = 0 if h == 0 else 119
    o_lo, o_hi = (0, 124) if h == 0 else (6, 130)
    inds = nlInds[f, :, 0]
    top = inds[:, 1].astype(np.int64)
    left = inds[:, 2].astype(np.int64)
    invperm = np.empty(P, np.int64)
    invperm[top * NPOS + left] = np.arange(P)
    sel = np.nonzero((top >= g0) & (top <= g0 + 130))[0]
    rank = np.full(P, -1, np.int64)
    rank[sel] = np.arange(NREAL)
    ar128 = np.arange(128)
    DM = -1
    gidx = np.zeros((SIN, 128), np.int64)
    s = 0
    for k in range(NB):
        for g in range(BT[k]):
            gt_ = g0 + SB[k] + g
            for ci, base in ((0, 0), (1, 122)):
                gidx[s] = rank[invperm[gt_ * NPOS + base + ar128]]
                s += 1
    sidx = np.full((SOUT, 128), DM, np.int64)
    s = 0
    for k in range(NB):
        t_lo, t_hi = OT[k]
        for lt in range(t_lo, t_hi + 1):
            gt_ = g0 + lt
            for ci, base in ((0, 0), (1, 122)):
                if o_lo <= lt <= o_hi:
                    col = rank[invperm[gt_ * NPOS + base + ar128]]
                    sidx[s] = col
                    if ci == 1:
                        sidx[s, :6] = DM
                s += 1
    # x plane: (128, SIN*147) dest-ordered slots, (j, i, c) element order, bf16
    import ml_dtypes
    xp = np.concatenate([x[f, sel, 0], np.zeros((1, D), np.float32)], axis=0)
    xs = xp[gidx.reshape(-1)][:, PELEM].reshape(SIN, 128, D) \
        .transpose(1, 0, 2).reshape(128, IN_W)
    x_bf = np.ascontiguousarray(xs.astype(ml_dtypes.bfloat16))
    # winv per (block, chunk): [128(par col), 126(rl, c)]
    winv = np.zeros((NB, 2, 128, RC), np.float32)
    for k in range(NB):
        for rl in range(RW):
            gr = g0 + SB[k] - 6 + rl
            lr = SB[k] - 6 + rl
            if 0 <= gr <= 255 and 0 <= lr <= 136:
                wr = _cntf(gr)
                for chunk in range(2):
                    cs = chunk * 128 + np.arange(128)
                    winv[k, chunk, :, rl * 3:rl * 3 + 3] = \
                        (1.0 / (wr * _cntf(cs)))[:, None]
    return dict(x_bf=x_bf,
                winv=np.ascontiguousarray(
                    winv.transpose(2, 0, 1, 3).reshape(128, NB * 2 * RC)),
                f=f, sel=sel, sidx=sidx)


def _identw():
    w = np.zeros((128, 262), np.float32)
    w[np.arange(128), np.arange(128) + 128] = 1.0
    return w


def _ap(base, off, dims):
    return bass.AP(base.tensor, base.offset + off,
                   [list(base.ap[0])] + [list(d) for d in dims])


def _mm(nc, out, lhsT, rhs, start, stop, ldw):
    """matmul with explicit ldweights control: ldw=False reuses the weights
    already loaded by the previous matmul in the same (chunk, j) group."""
    eng = nc.tensor
    ifmap_ap = eng.lower_ap(rhs.opt({0}), opt=False)
    weights_ap = eng.lower_ap(lhsT.opt({0}), opt=False, for_matmul_weights=True)
    out_ap = eng.lower_ap(out)
    return eng.add_instruction(mybir.InstMatmult(
        name=eng.bass.get_next_instruction_name(),
        replication_resolution=0,
        replication_shift_amnt=0,
        replication_num_rows=0,
        start_tensor_calc=start,
        stop_tensor_calc=stop,
        ins=[ifmap_ap, weights_ap],
        outs=[out_ap],
        perf_mode=None,
        is_transpose=False,
        ifmap_quant_offset=None,
        weights_quant_offset=None,
        bass_skip_group_check=None,
        tile_position=(lhsT.base_partition(), out.base_partition()),
        tile_size=(128, 128),
        ldweights=ldw,
    ))


LDW_DEDUP = True


def build_nc():
    nc = bacc.Bacc("TRN2", target_bir_lowering=False, debug=False, num_devices=8)
    xb_d = nc.declare_dram_parameter("x_bf", [128, IN_W], BF16, isOutput=False)
    ib_d = nc.declare_dram_parameter("identb", [128, 262], BF16, isOutput=False)
    wv_d = nc.declare_dram_parameter("winv", [128, NB * 2 * RC], F32,
                                     isOutput=False)
    id_d = nc.declare_dram_parameter("identw", [128, 262], F32, isOutput=False)
    y_d = nc.declare_dram_parameter("y_core", [128, OUT_W], BF16, isOutput=True)

    with tile.TileContext(nc) as tc:
        with tc.tile_pool(name="const", bufs=1) as cpool, \
             tc.tile_pool(name="gp", bufs=3) as gpool, \
             tc.tile_pool(name="vtp", bufs=2, space="PSUM") as vtps, \
             tc.tile_pool(name="vtu", bufs=2) as vtup, \
             tc.tile_pool(name="vbp", bufs=1, space="PSUM") as vbp, \
             tc.tile_pool(name="vsb", bufs=2) as vsbp, \
             tc.tile_pool(name="pjp", bufs=3, space="PSUM") as pjp, \
             tc.tile_pool(name="stg", bufs=2) as stgp:
            # identb + winv lead the SP ring (identb gates the first fold,
            # winv the first merge); identw (first needed by vb) and the
            # outputs ride the ACT ring so in/out streams don't serialize.
            identb = cpool.tile([128, 262], BF16)
            nc.sync.dma_start(out=identb[:], in_=ib_d[:])
            wvt = cpool.tile([128, NB * 2 * RC], F32)
            nc.sync.dma_start(out=wvt[:], in_=wv_d[:])
            identw = cpool.tile([128, 262], F32)
            nc.scalar.dma_start(out=identw[:], in_=id_d[:])

            vt_hist = [None] * NB
            in_off = [0]
            out_off = [0]

            def fold_block(k):
                G = BT[k]
                w = G * 2 * D
                gth = gpool.tile([128, w], BF16, tag="gth", name=f"gth{k}")
                nc.sync.dma_start(out=gth[:],
                                  in_=xb_d[:, in_off[0]: in_off[0] + w])
                in_off[0] += w
                r_last = min(6, G - 1)
                vts = []
                for chunk in range(2):
                    vt = vtps.tile([128, RC], F32, tag=f"vt{chunk}",
                                   name=f"vt{chunk}_{k}")
                    # g is split by residue mod 7 so one matmul's out runs
                    # (21m + 3i + c) never self-overlap; the fold overlap
                    # accumulates across matmuls via PSUM has_written.
                    for j in range(7):
                        d = j if chunk == 0 else j - 6
                        for r in range(7):
                            Gr = len(range(r, G, 7))
                            if Gr == 0:
                                continue
                            rhs = _ap(gth[:],
                                      chunk * D + r * 2 * D + j * 21,
                                      [(14 * D, Gr), (3, 7), (1, 3)])
                            out = _ap(vt[:], 3 * r,
                                      [(21, Gr), (3, 7), (1, 3)])
                            if LDW_DEDUP:
                                _mm(nc, out, identb[:, 128 - d:256 - d], rhs,
                                    start=(j == 0 and r == 0),
                                    stop=(j == 6 and r == r_last),
                                    ldw=(r == 0))
                            else:
                                nc.tensor.matmul(
                                    out, lhsT=identb[:, 128 - d:256 - d],
                                    rhs=rhs,
                                    start=(j == 0 and r == 0),
                                    stop=(j == 6 and r == r_last))
                    vts.append(vt)
                vt_hist[k] = vts
                if k >= 2:
                    vt_hist[k - 2] = None

            def unfold_block(k):
                G = BT[k]
                n1 = min(G, 30)              # rows copied from vt_k
                vtus = []
                for chunk in range(2):
                    vt = vt_hist[k][chunk]
                    vtu = vtup.tile([128, RC], F32, tag=f"vtu{chunk}",
                                    name=f"vtu{chunk}_{k}")
                    nc.vector.tensor_copy(out=vtu[:, 36:36 + 3 * n1],
                                          in_=vt[:, 18:18 + 3 * n1])
                    if k > 0:
                        Bp = BT[k - 1]
                        vtp = vt_hist[k - 1][chunk]
                        nc.vector.tensor_copy(
                            out=vtu[:, 0:36],
                            in_=vtp[:, 3 * (Bp - 6):3 * (Bp - 6) + 36])
                        nc.vector.tensor_tensor(
                            out=vtu[:, 18:36], in0=vtu[:, 18:36],
                            in1=vt[:, 0:18], op=mybir.AluOpType.add)
                    else:
                        nc.vector.tensor_copy(out=vtu[:, 18:36],
                                              in_=vt[:, 0:18])
                    wv = wvt[:, (k * 2 + chunk) * RC:
                             (k * 2 + chunk + 1) * RC]
                    nc.vector.tensor_tensor(out=vtu[:], in0=vtu[:], in1=wv,
                                            op=mybir.AluOpType.mult)
                    vtus.append(vtu)
                vb = vbp.tile([RC, 256], F32, tag="vb", name=f"vb{k}")
                for chunk in range(2):
                    nc.tensor.matmul(
                        vb[:, chunk * 128:(chunk + 1) * 128],
                        lhsT=vtus[chunk][:, 0:RC], rhs=identw[:, 128:256],
                        is_transpose=True,
                        start=(chunk == 0), stop=(chunk == 1))
                vsb = vsbp.tile([RC, 256], BF16, tag="vsb", name=f"vsb{k}")
                nc.scalar.copy(out=vsb[:], in_=vb[:])
                t_lo, t_hi = OT[k]
                nt = t_hi - t_lo + 1
                goff = t_lo - (SB[k] - 6)
                w = nt * 2 * D
                stg = stgp.tile([128, w], BF16, tag="stg", name=f"stg{k}")
                cpy = 0
                for j in range(7):
                    for ci, base in ((0, 0), (1, 122)):
                        pj = pjp.tile([128, RC], BF16, tag="pj",
                                      name=f"pj{k}_{j}_{ci}")
                        nc.tensor.matmul(
                            pj[:], lhsT=vsb[:, base + j: base + j + 128],
                            rhs=identb[0:RC, 128:128 + RC],
                            is_transpose=True, start=True, stop=True)
                        src = _ap(pj[:], goff * 3, [(3, nt), (3, 7), (1, 3)])
                        dst = _ap(stg[:], ci * D + j * 21,
                                  [(2 * D, nt), (3, 7), (1, 3)])
                        if cpy % 2 == 0:
                            nc.scalar.copy(out=dst, in_=src)
                        else:
                            nc.vector.tensor_copy(out=dst, in_=src)
                        cpy += 1
                nc.scalar.dma_start(out=y_d[:, out_off[0]: out_off[0] + w],
                                    in_=stg[:])
                out_off[0] += w

            for k in range(NB):
                fold_block(k)
                unfold_block(k)

    nc.compile()
    return nc


_NC_CACHE = [None]


def _build_in_maps(x, nlInds):
    cores = [_host_prep_core(x, nlInds, c) for c in range(8)]
    idw = _identw()
    import ml_dtypes
    idb = idw.astype(ml_dtypes.bfloat16)
    in_maps = [dict(x_bf=cr["x_bf"], winv=cr["winv"],
                    identw=idw, identb=idb) for cr in cores]
    return cores, in_maps


def kernel(x, nlDists, nlInds, pixels_h, pixels_w):
    x = np.ascontiguousarray(np.asarray(x, dtype=np.float32))
    nlInds = np.asarray(nlInds)
    if _NC_CACHE[0] is None:
        _NC_CACHE[0] = build_nc()
    nc = _NC_CACHE[0]
    cores, in_maps = _build_in_maps(x, nlInds)
    res = run_bass_kernel_spmd(nc, in_maps, list(range(8)))
    out = np.zeros((T, P, 1, 147), np.float32)
    for c in range(8):
        cr = cores[c]
        y = np.asarray(res.results[c]["y_core"]).astype(np.float32)
        ys = y.reshape(128, SOUT, D).transpose(1, 0, 2).reshape(-1, D)
        sidx = cr["sidx"].reshape(-1)
        valid = sidx >= 0
        out[cr["f"], cr["sel"][sidx[valid]], 0] = ys[valid][:, COLPERM]
    return out
